# revision 15
# baseline (speedup 1.0000x reference)
"""LorentzConv2d Trainium2 kernel (v7: host-padded inputs, fp8 DMA shifts,
fp8 DoubleRow box matmuls).

Full-input contract: kernel(x=[8,56,56,64], kernels=[64,64]) -> [8,56,56,64].
Data-parallel over batch: one image per NeuronCore (8 cores). Host prep is
layout/dtype only: zero-padded bf16 and fp8 copies of x, band matrices, and
the signed kernel matrix.

Per-core algorithm on the zero-padded 58x58 grid (pixel p = 58*gh+gw, SBUF
layout [128 part, 27 tiles, 64] with pixel = 128*t + part):
  u[p,o]  = sum_c x[p,c] gu_c k[o,c]  (PE bf16; col 64 gives sx = sum_{c>0} x)
  D[p,o]  = acosh(max(u,1+eps))^2 = ln(u+sqrt(u^2-1))^2    (ACT/DVE chain)
  D8 is staged to DRAM once; the 12 shifted operands (pixel+dlin,
  dlin = 58*dh+dw in [1,118]) arrive as plain offset DMA reads; shifted x
  comes straight from the host-padded fp8 input.
  G[p]    = sum_c gx[p,c]/512 * xsh[p,c]                   (DVE mul+reduce)
  F8[p,o] = D*Dsh*G -> fp8                                 (DVE/Pool/ACT)
  Q/512   = -box33(D^2/512) + 2 sum_d boxB_d(F8)   fp8 DoubleRow box matmuls
            with HALF-SHIFTED output tiles (out pixel = 128*t+64+i) so one
            256-row band (2 planes = field tiles t,t+1) covers offsets
            in [-60,60] in a single matmul per 8-tile chunk
  S1/8    = box33(D*sx/8)  (same machinery, separate PSUM, phase A)
  out_o   = (S1_psum*8/63) / sqrt(512*max(-Q_psum, 1e-8/512)) (o>0)
  out_0   = sqrt(1 + sum_o out_o^2)
fp8 fields are scaled to stay under the trn fp8e4 max-finite 240; the scales
fold exactly into the final constants. Validated end-to-end rel err ~1.3e-3.
"""

import os
import numpy as np

import bass_rust
import concourse.bass as bass
import concourse.bacc as bacc
import concourse.tile as tile
from concourse import mybir
from concourse.bass_utils import run_bass_kernel_spmd

F32 = mybir.dt.float32
BF16 = mybir.dt.bfloat16
FP8 = mybir.dt.float8e4
AF = mybir.ActivationFunctionType
OP = mybir.AluOpType
DR = mybir.MatmulPerfMode.DoubleRow

# geometry
H = W = 56
C = 64
O = 64
GW = 58
NG = GW * GW               # 3364
NT = 27                    # pixel tiles of 128
NP = NT * 128              # 3456 compute pixels
NFT = NT + 2               # field tiles incl. leading/trailing zero tile
NPAD8 = NP + 128           # padded fp8 plane rows (shift guard)
ACOSH_EPS = 1e-7
EPS_Q = 1e-8 / 512.0
OUT_SCALE = 8.0 / (63.0 * 512.0 ** 0.5)

DELTAS = [(0, 1), (0, 2), (1, -2), (1, -1), (1, 0), (1, 1), (1, 2),
          (2, -2), (2, -1), (2, 0), (2, 1), (2, 2)]
ND = len(DELTAS)
NB = 2 + ND                # diag, s1, delta-box bands
BI_DIAG = 0
BI_S1 = 1
BI_BOX0 = 2

CHUNKS = [(0, 8), (8, 8), (16, 8), (24, 3)]
GROUPS = [(0, 7), (7, 7), (14, 7), (21, 6)]
# delta groups with consecutive dlin: (first_delta_idx, count, base_dlin)
DGRPS = [(0, 2, 1), (2, 5, 56), (7, 5, 114)]


def grp_rhs(dram_ap, base, nd):
    """Merged shifted load src AP [128, NT, nd*64]: element (p, t, 64j+c) =
    dram row (base + j + 128*t + p), col c (adjacent rows are contiguous,
    so the delta dim folds into one fat inner run)."""
    ap = dram_ap[base:base + NP, :].rearrange("(t p) c -> p t c", p=128)
    ap.ap = bass_rust.VecI64Pair([(64, 128), (64 * 128, NT), (1, nd * 64)])
    return ap


def _interval(d):
    return range(max(-1, -1 - d), min(1, 1 - d) + 1)


def build_bands():
    """Box band matrices as [NB, 128(p), 2(plane), 128(i)] over a 256-pixel
    window starting one half-tile before the half-shifted out tile:
    T[64+i+s, i] = coeff for s in box."""
    b = np.zeros((NB, 256, 128), np.float32)
    box33 = [GW * a + bb for a in (-1, 0, 1) for bb in (-1, 0, 1)]

    def put_box(bi, offs, coeff):
        for i in range(128):
            for s in offs:
                b[bi, 64 + i + s, i] = coeff

    put_box(BI_DIAG, box33, -1.0)
    put_box(BI_S1, box33, 1.0)
    for di, (dh, dw) in enumerate(DELTAS):
        offs = [GW * a + bb for a in _interval(dh) for bb in _interval(dw)]
        put_box(BI_BOX0 + di, offs, 2.0)
    return np.ascontiguousarray(
        b.reshape(NB, 2, 128, 128).transpose(0, 2, 1, 3))


def dr_rhs(field, t0, tn):
    """Overlapping DoubleRow rhs [128, 2, tn, 64] over field tiles
    [t0, t0+tn]: plane k of out-tile t reads field tile t0+t+k."""
    ap = field[:, t0:t0 + tn + 1, :].unsqueeze(1).to_broadcast(
        [128, 2, tn + 1, 64])[:, :, 0:tn, :]
    ap.ap = bass_rust.VecI64Pair(
        [tuple(ap.ap[0]), (64, 2), (64, tn), (1, 64)])
    return ap


def build_nc():
    nc = bacc.Bacc(None)
    x16_in = nc.declare_dram_parameter("x16", [NP, C], BF16, isOutput=False)
    x8_in = nc.declare_dram_parameter("x8", [NPAD8, C], FP8, isOutput=False)
    gk_in = nc.declare_dram_parameter("gk16", [C, O + 1], BF16, isOutput=False)
    bands_in = nc.declare_dram_parameter("bands", [NB, 128, 2, 128], FP8,
                                         isOutput=False)
    id_in = nc.declare_dram_parameter("ident", [128, 128], BF16, isOutput=False)
    out_ext = nc.declare_dram_parameter("out", [H * W, O], F32, isOutput=True)

    def tiled(dram_ap, ntile=NT):
        return dram_ap[0:128 * ntile, :].rearrange("(t p) c -> p t c", p=128)

    with tile.TileContext(nc) as tc:
        with (
            tc.tile_pool(name="dram", bufs=1, space="DRAM") as dpool,
            tc.tile_pool(name="singles", bufs=1) as sg,
            tc.tile_pool(name="pp", bufs=1) as pp,
            tc.tile_pool(name="wk", bufs=2) as wk,
        ):
            dpadD = dpool.tile([NPAD8, O], FP8)
            opad = dpool.tile([NP, O], F32)

            # ---- constants + input into SBUF
            gk_sb = sg.tile([C, O + 1], BF16)
            nc.sync.dma_start(out=gk_sb[:], in_=gk_in[:])
            id_sb = sg.tile([128, 128], BF16)
            nc.sync.dma_start(out=id_sb[:], in_=id_in[:])
            bands_sb = sg.tile([128, NB, 2, 128], FP8)
            nc.sync.dma_start(out=bands_sb[:],
                              in_=bands_in.rearrange("b p two m -> p b two m"))
            z8 = sg.tile([128, O], FP8)
            nc.vector.memset(z8[:], 0.0)
            cneg1 = sg.tile([128, 1], F32)
            nc.vector.memset(cneg1[:], -1.0)

            x16 = sg.tile([128, NT, C], BF16)
            for g, (t0, tn) in enumerate(GROUPS):
                nc.sync.dma_start(
                    out=x16[:, t0:t0 + tn, :],
                    in_=x16_in[128 * t0:128 * (t0 + tn), :].rearrange(
                        "(t p) c -> p t c", p=128))
            # shifted-x loads straight from the host-padded fp8 plane,
            # one DMA per consecutive-dlin delta group
            xsh8 = []
            for gi, (d0, nd, base) in enumerate(DGRPS):
                tx = sg.tile([128, NT, nd * C], FP8, tag=f"xshg{gi}",
                             name=f"xshg{gi}")
                nc.sync.dma_start(out=tx[:], in_=grp_rhs(x8_in, base, nd))
                for j in range(nd):
                    xsh8.append(tx[:, :, C * j:C * (j + 1)])

            # ---- persistent fields
            gx16q = sg.tile([128, NT, C], BF16)   # g*x/512
            D8 = sg.tile([128, NT, O], FP8)
            D16 = sg.tile([128, NT, O], BF16)
            Fd8 = sg.tile([128, NFT, O], FP8)
            Fs8 = sg.tile([128, NFT, O], FP8)
            for f in (Fd8, Fs8):
                nc.vector.memset(f[:, 0, :], 0.0)
                nc.vector.memset(f[:, NFT - 1, :], 0.0)
            sx16 = sg.tile([128, NT], BF16)
            S1_16 = sg.tile([128, NT, O], BF16)
            xT16 = sg.tile([64, NT, 128], BF16)

            nc.scalar.activation(gx16q[:, :, 1:C], x16[:, :, 1:C], AF.Copy,
                                 scale=1.0 / 512.0)
            nc.scalar.activation(gx16q[:, :, 0], x16[:, :, 0], AF.Copy,
                                 scale=-1.0 / 512.0)

            # ================= phase A: u-matmuls + acosh chain ============
            with (
                tc.tile_pool(name="psT", bufs=3, space="PSUM") as psT,
                tc.tile_pool(name="psU", bufs=1, space="PSUM") as psU,
            ):
                psu_g = [psU.tile([128, 7, O + 1], F32, tag=f"psu{g}",
                                  name=f"psu{g}") for g in range(4)]
                for g, (t0, tn) in enumerate(GROUPS):
                    for i in range(tn):
                        tl = t0 + i
                        tp = psT.tile([64, 128], BF16)
                        nc.tensor.transpose(tp[:], x16[:, tl, :], id_sb[:])
                        nc.vector.tensor_copy(xT16[:, tl, :], tp[:])
                        nc.tensor.matmul(psu_g[g][:, i, :], xT16[:, tl, :],
                                         gk_sb[:], start=True, stop=True)
                um = pp.tile([128, NT, O], F32, tag="big0", name="um")
                sq = pp.tile([128, NT, O], F32, tag="big1", name="sq")
                rt = pp.tile([128, NT, O], F32, tag="big2", name="rt")
                vv = pp.tile([128, NT, O], F32, tag="big3", name="vv")
                lnv = pp.tile([128, NT, O], F32, tag="big4", name="lnv")

                def for_groups(fn):
                    for g, (t0, tn) in enumerate(GROUPS):
                        fn(g, slice(t0, t0 + tn), slice(1 + t0, 1 + t0 + tn),
                           tn)

                for_groups(lambda g, sl, fl, tn: nc.vector.tensor_scalar_max(
                    um[:, sl, :], psu_g[g][:, :tn, 0:O], 1.0 + ACOSH_EPS))
                for_groups(lambda g, sl, fl, tn: nc.scalar.copy(
                    sx16[:, sl], psu_g[g][:, :tn, O]))
                for_groups(lambda g, sl, fl, tn: nc.scalar.activation(
                    sq[:, sl, :], um[:, sl, :], AF.Square))
                for_groups(lambda g, sl, fl, tn: nc.scalar.activation(
                    rt[:, sl, :], sq[:, sl, :], AF.Sqrt, bias=cneg1[:]))
                for_groups(lambda g, sl, fl, tn: nc.gpsimd.tensor_add(
                    vv[:, sl, :], um[:, sl, :], rt[:, sl, :]))
                for_groups(lambda g, sl, fl, tn: nc.scalar.activation(
                    lnv[:, sl, :], vv[:, sl, :], AF.Ln))
                for_groups(lambda g, sl, fl, tn: nc.vector.tensor_mul(
                    D16[:, sl, :], lnv[:, sl, :], lnv[:, sl, :]))
                for_groups(lambda g, sl, fl, tn: nc.vector.tensor_copy(
                    D8[:, sl, :], D16[:, sl, :]))
                for_groups(lambda g, sl, fl, tn: nc.vector.scalar_tensor_tensor(
                    out=Fd8[:, fl, :], in0=D16[:, sl, :], scalar=1.0 / 512.0,
                    in1=D16[:, sl, :], op0=OP.mult, op1=OP.mult))
                for_groups(lambda g, sl, fl, tn: nc.vector.scalar_tensor_tensor(
                    out=Fs8[:, fl, :], in0=D16[:, sl, :], scalar=0.125,
                    in1=sx16[:, sl].unsqueeze(2).to_broadcast([128, tn, O]),
                    op0=OP.mult, op1=OP.mult))
                # D8 staging to DRAM + guard zeros
                nc.scalar.dma_start(out=dpadD[NP:NPAD8, :], in_=z8[:])
                nc.scalar.dma_start(out=tiled(dpadD), in_=D8[:])

            # shifted-D loads, one DMA per consecutive-dlin delta group
            dsh8 = []
            for gi, (d0, nd, base) in enumerate(DGRPS):
                td = sg.tile([128, NT, nd * O], FP8, tag=f"dshg{gi}",
                             name=f"dshg{gi}")
                nc.scalar.dma_start(out=td[:], in_=grp_rhs(dpadD, base, nd))
                for j in range(nd):
                    dsh8.append(td[:, :, O * j:O * (j + 1)])

            # ================= phase A2: S1 box =================
            with tc.tile_pool(name="psS", bufs=1, space="PSUM") as psS:
                ps_s = psS.tile([128, NT, O], F32)
                for (c0, cw) in CHUNKS:
                    nc.tensor.matmul(ps_s[:, c0:c0 + cw, :],
                                     bands_sb[:, BI_S1, :, :],
                                     dr_rhs(Fs8, c0, cw),
                                     start=True, stop=True, perf_mode=DR,
                                     skip_group_check=True)
                    nc.scalar.copy(S1_16[:, c0:c0 + cw, :],
                                   ps_s[:, c0:c0 + cw, :])

            # ================= phase C: deltas =================
            with (
                tc.tile_pool(name="psQ", bufs=1, space="PSUM") as psQp,
                tc.tile_pool(name="f8p", bufs=3) as f8p,
            ):
                ps_q = psQp.tile([128, NT, O], F32)
                wq = [0] * len(CHUNKS)
                NWQ = 1 + ND

                def box_pass(bi, field):
                    for ci, (c0, cw) in enumerate(CHUNKS):
                        nc.tensor.matmul(ps_q[:, c0:c0 + cw, :],
                                         bands_sb[:, bi, :, :],
                                         dr_rhs(field, c0, cw),
                                         start=(wq[ci] == 0),
                                         stop=(wq[ci] == NWQ - 1),
                                         perf_mode=DR, skip_group_check=True)
                        wq[ci] += 1

                box_pass(BI_DIAG, Fd8)

                f8_bufs = []
                for i in range(3):
                    f = f8p.tile([128, NFT, O], FP8, tag=f"f8_{i}",
                                 name=f"f8_{i}")
                    nc.vector.memset(f[:, 0, :], 0.0)
                    nc.vector.memset(f[:, NFT - 1, :], 0.0)
                    f8_bufs.append(f)

                with nc.allow_low_precision(reason="G in bf16 is plenty"):
                    prev = None
                    for d in range(ND):
                        F8f = f8_bufs[d % 3]
                        gxs = wk.tile([128, NT, C], BF16, tag="gxs",
                                      name=f"gxs{d}")
                        geng = nc.vector if d % 2 == 0 else nc.gpsimd
                        geng.tensor_mul(gxs[:], gx16q[:], xsh8[d])
                        G16 = wk.tile([128, NT], BF16, tag="G16",
                                      name=f"G16{d}")
                        nc.vector.tensor_reduce(G16[:], gxs[:],
                                                axis=mybir.AxisListType.X,
                                                op=OP.add)
                        G64 = wk.tile([128, NT, O], BF16, tag="G64",
                                      name=f"G64{d}")
                        nc.scalar.copy(
                            G64[:], G16[:].unsqueeze(2).to_broadcast(
                                [128, NT, O]))
                        t2 = wk.tile([128, NT, O], BF16, tag="t2",
                                     name=f"t2{d}")
                        teng = nc.gpsimd if d % 2 == 0 else nc.vector
                        teng.tensor_mul(t2[:], D16[:], dsh8[d])
                        nc.vector.tensor_mul(F8f[:, 1:NT + 1, :], t2[:],
                                             G64[:])
                        if prev is not None:
                            box_pass(BI_BOX0 + d - 1, prev)
                        prev = F8f
                    box_pass(BI_BOX0 + ND - 1, prev)

                # ================= phase D: normalize & emit ===============
                osb = pp.tile([128, NT, O], F32, tag="big0", name="osb")
                negq = pp.tile([128, NT, O], F32, tag="big1", name="nq")
                lncl = pp.tile([128, NT, O], F32, tag="big2", name="lncl")
                rr = pp.tile([128, NT, O], F32, tag="big3", name="rr")
                s2 = pp.tile([128, NT, O - 1], F32, tag="big4", name="s2")
                red = pp.tile([128, NT], F32, tag="red", name="red")
                DCH = ((0, 14), (14, 13))
                for (c0, cw) in DCH:
                    sl = slice(c0, c0 + cw)
                    nc.vector.tensor_scalar(negq[:, sl, :], ps_q[:, sl, :],
                                            -1.0, EPS_Q, op0=OP.mult,
                                            op1=OP.max)
                for (c0, cw) in DCH:
                    sl = slice(c0, c0 + cw)
                    nc.scalar.activation(lncl[:, sl, :], negq[:, sl, :], AF.Ln)
                for (c0, cw) in DCH:
                    sl = slice(c0, c0 + cw)
                    nc.scalar.activation(rr[:, sl, :], lncl[:, sl, :],
                                         AF.Exp, scale=-0.5)
                    nc.vector.scalar_tensor_tensor(
                        out=osb[:, sl, :], in0=S1_16[:, sl, :],
                        scalar=OUT_SCALE, in1=rr[:, sl, :],
                        op0=OP.mult, op1=OP.mult)
                    nc.vector.tensor_mul(s2[:, sl, :], osb[:, sl, 1:O],
                                         osb[:, sl, 1:O])
                    nc.vector.tensor_reduce(red[:, sl], s2[:, sl, :],
                                            axis=mybir.AxisListType.X,
                                            op=OP.add)
                for (c0, cw) in DCH:
                    sl = slice(c0, c0 + cw)
                    nc.scalar.activation(osb[:, sl, 0], red[:, sl], AF.Sqrt,
                                         bias=1.0)
                    nc.sync.dma_start(out=tiled(opad)[:, sl, :],
                                      in_=osb[:, sl, :])

            # interior extraction: out pixel q lives at opad row q + 64
            nc.sync.dma_start(
                out=out_ext.rearrange("(h w) c -> h w c", w=W),
                in_=opad[123:123 + 56 * GW, :].rearrange(
                    "(h r) c -> h r c", r=GW)[:, 0:56, :])
    nc.finalize()
    return nc


_NC_CACHE = None


def _get_nc():
    global _NC_CACHE
    if _NC_CACHE is None:
        _NC_CACHE = build_nc()
    return _NC_CACHE


def host_consts(kernels):
    import ml_dtypes
    gk = np.zeros((C, O + 1), np.float32)
    gk[:, :O] = kernels.astype(np.float32).T
    gk[1:, :O] *= -1.0
    gk[1:, O] = 1.0
    gk16 = np.ascontiguousarray(gk.astype(ml_dtypes.bfloat16))
    bands8 = np.ascontiguousarray(build_bands().astype(ml_dtypes.float8_e4m3fn))
    ident16 = np.ascontiguousarray(np.eye(128).astype(ml_dtypes.bfloat16))
    return gk16, bands8, ident16


def host_pad(xi):
    """Zero-padded 58x58 grid planes of one image: bf16 [NP, C] and
    fp8 [NPAD8, C] (with shift guard rows)."""
    import ml_dtypes
    xp = np.zeros((NPAD8, C), np.float32)
    xg = np.zeros((GW, GW, C), np.float32)
    xg[1:57, 1:57] = xi
    xp[:NG] = xg.reshape(NG, C)
    x16 = np.ascontiguousarray(xp[:NP].astype(ml_dtypes.bfloat16))
    x8 = np.ascontiguousarray(
        x16.astype(np.float32).astype(ml_dtypes.float8_e4m3fn))
    x8 = np.concatenate(
        [x8, np.zeros((NPAD8 - NP, C), ml_dtypes.float8_e4m3fn)], 0)
    return x16, np.ascontiguousarray(x8)


def kernel(x, kernels):
    x = np.asarray(x, dtype=np.float32)
    kernels = np.asarray(kernels, dtype=np.float32)
    B = x.shape[0]
    assert x.shape == (B, H, W, C) and B == 8, x.shape
    gk16, bands8, ident16 = host_consts(kernels)
    nc = _get_nc()
    in_maps = []
    for i in range(8):
        x16, x8 = host_pad(x[i])
        in_maps.append({"x16": x16, "x8": x8, "gk16": gk16,
                        "bands": bands8, "ident": ident16})
    res = run_bass_kernel_spmd(nc, in_maps, core_ids=list(range(8)),
                               trace=bool(int(os.environ.get("KTRACE", "0"))))
    if res.exec_time_ns is not None:
        print(f"HW exec time: {res.exec_time_ns} ns")
    out = np.stack([res.results[i]["out"].reshape(H, W, O) for i in range(8)])
    return out.astype(np.float32)
